# revision 1
# baseline (speedup 1.0000x reference)
"""Trainium2 kernel for AutoPatchOverLapModel3D (3D patch overlap-add / fold).

Math: out[b,p,y0,y1,y2] = (1/CM[y0,y1,y2]) * sum_{j0,j1,j2}
        x[b, y0-j0, y1-j1, (y2-j2)%64, p, j0, j1, j2]
i.e. a stride-1 overlap-add of 5x5x5 patches; axes 0/1 zero-padded,
axis 2 circular; CM is the separable patch-count normalizer.

Strategy (8 NeuronCores, SPMD):
  - The patch index n factors as n = col*64 + i2 with col=(b*10+i0)*28+i1
    (560 columns of 64 circularly-coupled patches each). Shard 70
    columns per core: each core reads a contiguous 44.8 MB slice.
  - On device, fold the circular i2/j2 axis with TensorE matmuls:
    128 patches (2 columns) per group on the contraction axis, using a
    block-diagonal 0/1 shift-weight matrix per j2 tap (5 taps
    accumulated in PSUM).  out_col[y2, (p,j0,j1)] per column.
  - The tiny j0/j1 overlap-add across columns (5x5 shifted adds of a
    4.6 MB result) and the CM division run on the host.
"""

import numpy as np

B, X0, X1, X2, P = 2, 10, 28, 64, 20
PK = 5  # patch edge
Y0, Y1, Y2 = 14, 32, 64
NCOL = B * X0 * X1            # 560 (b,i0,i1) columns
NCORES = 8
COLS_PER_CORE = NCOL // NCORES  # 70
PATCH_VEC = P * PK * PK * PK    # 2500
FREE = P * PK * PK              # 500 = (p, j0, j1)
GROUPS = COLS_PER_CORE * X2 // 128  # 35 groups of 128 patches (2 cols)
FRAMES = 5                      # half-plane frames per core (14 cols each)
GROUPS_PER_FRAME = 7
YF = 18                         # y1 span within a half-plane: 13 + 4 + 1

_CACHE = {}


def _shift_weights():
    # w[k, j2*128 + m]: k = u*64 + i2, m = u*64 + y2 ;  1.0 iff same u
    # and y2 == (i2 + j2 - 2) % 64 (the reference's circular axis keeps
    # patch centers at their own index: tap j2 lands at offset j2-2).
    # Block-diagonal over the 2 columns that share a matmul group.
    w = np.zeros((128, 5, 128), np.float32)
    i2 = np.arange(64)
    for j2 in range(5):
        y2 = (i2 + j2 - 2) % 64
        for u in range(2):
            w[u * 64 + i2, j2, u * 64 + y2] = 1.0
    return w.reshape(128, 5 * 128)


def _kernel_body(tc, xs, w, out):
    import concourse.mybir as mybir

    nc = tc.nc
    f32 = mybir.dt.float32
    f32r = xs.dtype  # float32r on HW (fast fp32 matmul path), f32 in sim
    with (
        tc.tile_pool(name="wpool", bufs=1) as wpool,
        tc.tile_pool(name="xpool", bufs=8) as xpool,
        tc.tile_pool(name="accpool", bufs=3) as accpool,
        tc.tile_pool(name="pspool", bufs=6, space="PSUM") as pspool,
    ):
        wt = wpool.tile([128, 5 * 128], f32r)
        nc.sync.dma_start(out=wt[:, :], in_=w[:, :])
        # 5 half-plane frames of 7 groups (14 columns) each; frame
        # boundaries are half-plane aligned on every core (70 % 14 == 0),
        # keeping the program SPMD-uniform.
        for h in range(FRAMES):
            acc = accpool.tile([128, 100 * YF], f32)
            nc.gpsimd.memset(acc[:, :], 0.0)
            av = acc[:, :].rearrange("a (f y) -> a y f", y=YF)
            for q in range(GROUPS_PER_FRAME):
                g = h * GROUPS_PER_FRAME + q
                xt = xpool.tile([128, PATCH_VEC], f32r)
                nc.sync.dma_start(
                    out=xt[:, :], in_=xs[g * 128:(g + 1) * 128, :]
                )
                ps = pspool.tile([128, FREE], f32)
                xv = xt[:, :].rearrange("a (f j) -> a j f", j=5)
                for j2 in range(5):
                    nc.tensor.matmul(
                        ps[:, :],
                        wt[:, j2 * 128:(j2 + 1) * 128],
                        xv[:, j2, :],
                        start=(j2 == 0),
                        stop=(j2 == 4),
                    )
                # fold j1 on-device: column i1 = 2q+u lands at y1f = i1+j1.
                # One 3D-AP add per u-block covers all 5 j1 taps at once
                # (dst y1f window [2q+u, 2q+u+5) is stride-1, like j1).
                pv = ps[:, :].rearrange("a (f j) -> a j f", j=5)
                for u in range(2):
                    lo = 2 * q + u
                    dst = av[u * 64:(u + 1) * 64, lo:lo + 5, :]
                    nc.vector.tensor_add(
                        dst, dst, pv[u * 64:(u + 1) * 64, :, :]
                    )
            nc.gpsimd.dma_start(out=out[h, :, :], in_=acc[:, :])


def _build_nc():
    import concourse.bacc as bacc
    import concourse.mybir as mybir
    import concourse.tile as tile

    nc = bacc.Bacc(
        "TRN2",
        target_bir_lowering=False,
        debug=False,
        enable_asserts=True,
        num_devices=NCORES,
    )
    f32 = mybir.dt.float32
    xs = nc.declare_dram_parameter("xs", [COLS_PER_CORE * 64, PATCH_VEC], mybir.dt.float32r, isOutput=False)
    w = nc.declare_dram_parameter("w", [128, 5 * 128], mybir.dt.float32r, isOutput=False)
    out = nc.declare_dram_parameter("out", [FRAMES, 128, 100 * YF], f32, isOutput=True)

    with tile.TileContext(nc) as tc:
        _kernel_body(tc, xs, w, out)
    nc.compile()
    return nc


def _counting_matrix():
    c0 = np.zeros(Y0, np.float32)
    for i0 in range(X0):
        c0[i0:i0 + PK] += 1
    c1 = np.zeros(Y1, np.float32)
    for i1 in range(X1):
        c1[i1:i1 + PK] += 1
    return c0[:, None, None] * c1[None, :, None] * 5.0


def kernel(x: np.ndarray) -> np.ndarray:
    from concourse.bass_utils import run_bass_kernel_spmd

    if "nc" not in _CACHE:
        _CACHE["nc"] = _build_nc()
    nc = _CACHE["nc"]

    xf = np.ascontiguousarray(x, np.float32).reshape(NCOL * X2, PATCH_VEC)
    wnp = _shift_weights()
    rows = COLS_PER_CORE * X2
    in_maps = [
        {"xs": xf[c * rows:(c + 1) * rows], "w": wnp} for c in range(NCORES)
    ]
    res = run_bass_kernel_spmd(nc, in_maps, list(range(NCORES)))
    oc = np.stack([res.results[c]["out"] for c in range(NCORES)], axis=0)

    # host stitch: oc[c, h] holds half-plane H=5c+h partials
    # [(u, y2), (p, j0, y1f)]; place at y1 = 14*(H%2) + y1f, y0 = i0 + j0.
    ocr = oc.reshape(NCORES * FRAMES, 2, 64, P, PK, YF)     # H,u,y2,p,j0,y1f
    ocr = ocr.sum(1).transpose(0, 2, 3, 4, 1)               # H,p,j0,y1f,y2
    out = np.zeros((B, P, Y0, Y1, Y2), np.float32)
    for H in range(NCORES * FRAMES):
        gp, half = divmod(H, 2)
        b, i0 = divmod(gp, X0)
        y1lo = (X1 // 2) * half
        out[b, :, i0:i0 + PK, y1lo:y1lo + YF, :] += ocr[H]
    out /= _counting_matrix()
    return out



# revision 2
# speedup vs baseline: 1.2124x; 1.2124x over previous
"""Trainium2 kernel for AutoPatchOverLapModel3D (3D patch overlap-add / fold).

Math: out[b,p,y0,y1,y2] = (1/CM[y0,y1,y2]) * sum_{j0,j1,j2}
        x[b, y0-j0, y1-j1, (y2-j2)%64, p, j0, j1, j2]
i.e. a stride-1 overlap-add of 5x5x5 patches; axes 0/1 zero-padded,
axis 2 circular; CM is the separable patch-count normalizer.

Strategy (8 NeuronCores, SPMD):
  - The patch index n factors as n = col*64 + i2 with col=(b*10+i0)*28+i1
    (560 columns of 64 circularly-coupled patches each). Shard 70
    columns per core: each core reads a contiguous slice.
  - Inputs are cast to fp16 on the host: halves HBM traffic and the
    matmul runs the full-rate 1-cycle/row PE path (PSUM accumulation
    stays fp32, the shift weights are exact 0/1, so only the input
    quantization (~5e-4 rel) is lost — far inside the 2e-2 gate).
  - On device, fold the circular i2/j2 axis with TensorE matmuls:
    128 patches (2 columns) per group on the contraction axis, using a
    block-diagonal 0/1 shift-weight matrix per j2 tap (5 taps
    accumulated in PSUM).  out_col[y2, (p,j0,j1)] per column.
  - The j1 fold runs on-device as one 128-partition DVE add per group
    into a u-shifted accumulator (u=1 partials stored at y1f-1, fixed
    up on the host), so both u-blocks fold in a single instruction.
  - The tiny j0 overlap-add across planes and the CM division run on
    the host.
"""

import numpy as np

B, X0, X1, X2, P = 2, 10, 28, 64, 20
PK = 5  # patch edge
Y0, Y1, Y2 = 14, 32, 64
NCOL = B * X0 * X1            # 560 (b,i0,i1) columns
NCORES = 8
COLS_PER_CORE = NCOL // NCORES  # 70
PATCH_VEC = P * PK * PK * PK    # 2500
FREE = P * PK * PK              # 500 = (p, j0, j1)
GROUPS = COLS_PER_CORE * X2 // 128  # 35 groups of 128 patches (2 cols)
FRAMES = 5                      # half-plane frames per core (14 cols each)
GROUPS_PER_FRAME = 7
YF = 17                         # y1f span stored per frame: 2q+j1, q<7

_CACHE = {}


def _shift_weights():
    # w[k, j2*128 + m]: k = u*64 + i2, m = u*64 + y2 ;  1.0 iff same u
    # and y2 == (i2 + j2 - 2) % 64 (the reference's circular axis keeps
    # patch centers at their own index: tap j2 lands at offset j2-2).
    # Block-diagonal over the 2 columns that share a matmul group.
    w = np.zeros((128, 5, 128), np.float16)
    i2 = np.arange(64)
    for j2 in range(5):
        y2 = (i2 + j2 - 2) % 64
        for u in range(2):
            w[u * 64 + i2, j2, u * 64 + y2] = 1.0
    return w.reshape(128, 5 * 128)


def _kernel_body(tc, xs, w, out):
    import concourse.mybir as mybir

    nc = tc.nc
    f32 = mybir.dt.float32
    f16 = xs.dtype
    with (
        tc.tile_pool(name="wpool", bufs=1) as wpool,
        tc.tile_pool(name="xpool", bufs=8) as xpool,
        tc.tile_pool(name="accpool", bufs=3) as accpool,
        tc.tile_pool(name="pspool", bufs=6, space="PSUM") as pspool,
    ):
        wt = wpool.tile([128, 5 * 128], f16)
        nc.sync.dma_start(out=wt[:, :], in_=w[:, :])
        # 5 half-plane frames of 7 groups (14 columns) each; frame
        # boundaries are half-plane aligned on every core (70 % 14 == 0),
        # keeping the program SPMD-uniform.
        for h in range(FRAMES):
            acc = accpool.tile([128, 100 * YF], f32)
            nc.gpsimd.memset(acc[:, :], 0.0)
            av = acc[:, :].rearrange("a (y f) -> a y f", y=YF)
            for q in range(GROUPS_PER_FRAME):
                g = h * GROUPS_PER_FRAME + q
                xt = xpool.tile([128, PATCH_VEC], f16)
                # alternate input loads across two DMA rings
                eng = nc.sync if (g % 2 == 0) else nc.scalar
                eng.dma_start(
                    out=xt[:, :], in_=xs[g * 128:(g + 1) * 128, :]
                )
                ps = pspool.tile([128, FREE], f32)
                xv = xt[:, :].rearrange("a (f j) -> a j f", j=5)
                for j2 in range(5):
                    nc.tensor.matmul(
                        ps[:, :],
                        wt[:, j2 * 128:(j2 + 1) * 128],
                        xv[:, j2, :],
                        start=(j2 == 0),
                        stop=(j2 == 4),
                    )
                # fold j1 on-device: column i1 = 2q+u lands at stored row
                # y1f = 2q+j1 for BOTH u-blocks (u=1 is stored shifted by
                # -1 and fixed up on the host), so one 128-partition 3D-AP
                # add covers the whole group.
                pv = ps[:, :].rearrange("a (f j) -> a j f", j=5)
                lo = 2 * q
                dst = av[:, lo:lo + 5, :]
                nc.vector.tensor_add(dst, dst, pv[:, :, :])
            nc.gpsimd.dma_start(out=out[h, :, :], in_=acc[:, :])


def _build_nc():
    import concourse.bacc as bacc
    import concourse.mybir as mybir
    import concourse.tile as tile

    nc = bacc.Bacc(
        "TRN2",
        target_bir_lowering=False,
        debug=False,
        enable_asserts=True,
        num_devices=NCORES,
    )
    f32 = mybir.dt.float32
    f16 = mybir.dt.float16
    xs = nc.declare_dram_parameter("xs", [COLS_PER_CORE * 64, PATCH_VEC], f16, isOutput=False)
    w = nc.declare_dram_parameter("w", [128, 5 * 128], f16, isOutput=False)
    out = nc.declare_dram_parameter("out", [FRAMES, 128, 100 * YF], f32, isOutput=True)

    with tile.TileContext(nc) as tc:
        _kernel_body(tc, xs, w, out)
    nc.compile()
    return nc


def _counting_matrix():
    c0 = np.zeros(Y0, np.float32)
    for i0 in range(X0):
        c0[i0:i0 + PK] += 1
    c1 = np.zeros(Y1, np.float32)
    for i1 in range(X1):
        c1[i1:i1 + PK] += 1
    return c0[:, None, None] * c1[None, :, None] * 5.0


def kernel(x: np.ndarray) -> np.ndarray:
    from concourse.bass_utils import run_bass_kernel_spmd

    if "nc" not in _CACHE:
        _CACHE["nc"] = _build_nc()
    nc = _CACHE["nc"]

    xf = np.ascontiguousarray(
        x, np.float32).reshape(NCOL * X2, PATCH_VEC).astype(np.float16)
    wnp = _shift_weights()
    rows = COLS_PER_CORE * X2
    in_maps = [
        {"xs": xf[c * rows:(c + 1) * rows], "w": wnp} for c in range(NCORES)
    ]
    res = run_bass_kernel_spmd(nc, in_maps, list(range(NCORES)))
    oc = np.stack([res.results[c]["out"] for c in range(NCORES)], axis=0)

    # host stitch: oc[c, h] holds half-plane H=5c+h partials
    # [(u, y2), (y1f, p, j0)]; place at y1 = 14*(H%2) + u + y1f,
    # y0 = i0 + j0.
    ocr = oc.reshape(NCORES * FRAMES, 2, 64, YF, P, PK)     # H,u,y2,y1f,p,j0
    ocr = ocr.transpose(0, 1, 4, 5, 3, 2)                   # H,u,p,j0,y1f,y2
    out = np.zeros((B, P, Y0, Y1, Y2), np.float32)
    for H in range(NCORES * FRAMES):
        gp, half = divmod(H, 2)
        b, i0 = divmod(gp, X0)
        y1lo = (X1 // 2) * half
        for u in range(2):
            out[b, :, i0:i0 + PK, y1lo + u:y1lo + u + YF, :] += ocr[H, u]
    out /= _counting_matrix()
    return out


# revision 4
# speedup vs baseline: 1.6804x; 1.3860x over previous
"""Trainium2 kernel for AutoPatchOverLapModel3D (3D patch overlap-add / fold).

Math: out[b,p,y0,y1,y2] = (1/CM[y0,y1,y2]) * sum_{j0,j1,j2}
        x[b, y0-j0, y1-j1, (y2-j2)%64, p, j0, j1, j2]
i.e. a stride-1 overlap-add of 5x5x5 patches; axes 0/1 zero-padded,
axis 2 circular; CM is the separable patch-count normalizer.

Strategy (8 NeuronCores, SPMD):
  - The patch index n factors as n = col*64 + i2 with col=(b*10+i0)*28+i1
    (560 columns of 64 circularly-coupled patches each). Shard 70
    columns per core; groups of 128 patches (2 columns) per tile.
  - The circular j2 fold is a +/-2 partition rotation. Rather than
    matmul against shift matrices, the HOST pre-rotates each of the 5
    j2 tap blocks (a free numpy permutation while casting to fp16), so
    on device every tap is partition-aligned and the fold is a plain
    5-way elementwise sum, split across engines to stay under the DMA
    roofline: 3 blocks summed on TensorE via identity-weight matmuls
    accumulating in PSUM (full-rate fp16, 1 cycle/row), 2 blocks added
    directly by the DVE in its 2x fp16 mode.
  - The j1 fold lands in a per-frame fp16 accumulator (one DVE add per
    group into a y1f window; u=1 columns stored shifted by -1 and fixed
    up on the host), fp16 halving both DVE cost and output traffic.
  - The tiny j0 overlap-add across planes and the CM division run on
    the host. Input quantization fp32->fp16 costs ~3e-4 rel error,
    far inside the 2e-2 gate.
"""

import numpy as np

B, X0, X1, X2, P = 2, 10, 28, 64, 20
PK = 5  # patch edge
Y0, Y1, Y2 = 14, 32, 64
NCOL = B * X0 * X1            # 560 (b,i0,i1) columns
NCORES = 8
COLS_PER_CORE = NCOL // NCORES  # 70
PATCH_VEC = P * PK * PK * PK    # 2500
FREE = P * PK * PK              # 500 = (p, j0, j1)
NGROUPS = NCOL * X2 // 128      # 280 groups of 128 patches (2 cols)
GROUPS = COLS_PER_CORE * X2 // 128  # 35 groups per core
FRAMES = 5                      # half-plane frames per core (14 cols each)
GROUPS_PER_FRAME = 7
YF = 17                         # y1f span stored per frame: 2q+j1, q<7

_CACHE = {}


def _prepare_inputs(x):
    """Cast to fp16 and pre-rotate the five j2 tap blocks.

    Returns per-core input dicts. Block k of 500 (p,j0,j1) floats holds
    tap j2=k rotated so SBUF partition m=(u,y2) already contains
    x[col(u), i2=(y2-k+2)%64, :, :, :, k]; the on-device j2 fold is
    then a plain 5-way aligned sum.
    """
    xf = np.ascontiguousarray(x, np.float32).astype(np.float16)
    A = xf.reshape(NCOL * X2, P, PK, PK, PK)
    A = A.transpose(0, 4, 3, 1, 2)                # n, j2, j1, p, j0
    A = A.reshape(NGROUPS, 2, 64, PK, FREE)       # g, u, i2, j2, (j1 p j0)
    Bt = np.empty((NGROUPS, 2, 64, PATCH_VEC), np.float16)
    for j2 in range(PK):
        Bt[:, :, :, j2 * FREE:(j2 + 1) * FREE] = np.roll(
            A[:, :, :, j2, :], j2 - 2, axis=2
        )
    Bt = Bt.reshape(NGROUPS, 128, PATCH_VEC)
    wnp = np.eye(128, dtype=np.float16)
    return [
        {"xs": Bt[c * GROUPS:(c + 1) * GROUPS].reshape(GROUPS * 128, PATCH_VEC),
         "w": wnp}
        for c in range(NCORES)
    ]


def _kernel_body(tc, xs, w, out):
    import concourse.mybir as mybir

    nc = tc.nc
    f16 = xs.dtype
    f32 = mybir.dt.float32
    with (
        tc.tile_pool(name="wpool", bufs=1) as wpool,
        tc.tile_pool(name="xpool", bufs=8) as xpool,
        tc.tile_pool(name="accpool", bufs=3) as accpool,
        tc.tile_pool(name="pspool", bufs=6, space="PSUM") as pspool,
    ):
        wt = wpool.tile([128, 128], f16)
        nc.sync.dma_start(out=wt[:, :], in_=w[:, :])
        # 5 half-plane frames of 7 groups (14 columns) each; frame
        # boundaries are half-plane aligned on every core (70 % 14 == 0),
        # keeping the program SPMD-uniform.
        for h in range(FRAMES):
            acc = accpool.tile([128, 100 * YF], f16)
            nc.scalar.memzero(acc[:, :])
            av = acc[:, :].rearrange("a (y f) -> a y f", y=YF)
            for q in range(GROUPS_PER_FRAME):
                g = h * GROUPS_PER_FRAME + q
                xt = xpool.tile([128, PATCH_VEC], f16)
                # alternate input loads across the two HWDGE rings
                eng = nc.sync if (g % 2 == 0) else nc.scalar
                eng.dma_start(
                    out=xt[:, :], in_=xs[g * 128:(g + 1) * 128, :]
                )
                # j2 fold, taps 0-2: identity-weight matmuls accumulate
                # the pre-rotated blocks in PSUM at full fp16 rate.
                ps = pspool.tile([128, FREE], f32)
                for k in range(3):
                    nc.tensor.matmul(
                        ps[:, :],
                        wt[:, :],
                        xt[:, k * FREE:(k + 1) * FREE],
                        start=(k == 0),
                        stop=(k == 2),
                    )
                # j1 fold: column i1 = 2q+u lands at stored row y1f =
                # 2q+j1 for BOTH u-blocks (u=1 stored shifted by -1 and
                # fixed up on the host). Taps 3-4 of the j2 fold are
                # added straight from xt by the DVE (fp16 2x mode).
                lo = 2 * q
                dst = av[:, lo:lo + 5, :]
                pv = ps[:, :].rearrange("a (j f) -> a j f", j=5)
                nc.vector.tensor_add(dst, dst, pv[:, :, :])
                for k in (3, 4):
                    tv = xt[:, k * FREE:(k + 1) * FREE].rearrange(
                        "a (j f) -> a j f", j=5
                    )
                    nc.vector.tensor_add(dst, dst, tv[:, :, :])
            nc.gpsimd.dma_start(out=out[h, :, :], in_=acc[:, :])


def _build_nc():
    import concourse.bacc as bacc
    import concourse.mybir as mybir
    import concourse.tile as tile

    nc = bacc.Bacc(
        "TRN2",
        target_bir_lowering=False,
        debug=False,
        enable_asserts=True,
        num_devices=NCORES,
    )
    f16 = mybir.dt.float16
    xs = nc.declare_dram_parameter("xs", [GROUPS * 128, PATCH_VEC], f16, isOutput=False)
    w = nc.declare_dram_parameter("w", [128, 128], f16, isOutput=False)
    out = nc.declare_dram_parameter("out", [FRAMES, 128, 100 * YF], f16, isOutput=True)

    with tile.TileContext(nc) as tc:
        _kernel_body(tc, xs, w, out)
    nc.compile()
    return nc


def _counting_matrix():
    c0 = np.zeros(Y0, np.float32)
    for i0 in range(X0):
        c0[i0:i0 + PK] += 1
    c1 = np.zeros(Y1, np.float32)
    for i1 in range(X1):
        c1[i1:i1 + PK] += 1
    return c0[:, None, None] * c1[None, :, None] * 5.0


def _stitch(oc):
    # oc[c, h] holds half-plane H=5c+h partials [(u, y2), (y1f, p, j0)];
    # place at y1 = 14*(H%2) + u + y1f, y0 = i0 + j0.
    ocr = oc.reshape(NCORES * FRAMES, 2, 64, YF, P, PK).astype(np.float32)
    ocr = ocr.transpose(0, 1, 4, 5, 3, 2)                   # H,u,p,j0,y1f,y2
    out = np.zeros((B, P, Y0, Y1, Y2), np.float32)
    for H in range(NCORES * FRAMES):
        gp, half = divmod(H, 2)
        b, i0 = divmod(gp, X0)
        y1lo = (X1 // 2) * half
        for u in range(2):
            out[b, :, i0:i0 + PK, y1lo + u:y1lo + u + YF, :] += ocr[H, u]
    out /= _counting_matrix()
    return out


def kernel(x: np.ndarray) -> np.ndarray:
    from concourse.bass_utils import run_bass_kernel_spmd

    if "nc" not in _CACHE:
        _CACHE["nc"] = _build_nc()
    nc = _CACHE["nc"]

    in_maps = _prepare_inputs(x)
    res = run_bass_kernel_spmd(nc, in_maps, list(range(NCORES)))
    oc = np.stack([res.results[c]["out"] for c in range(NCORES)], axis=0)
    return _stitch(oc)
